# revision 24
# baseline (speedup 1.0000x reference)
"""Trainium2 Bass kernel for the branched cross-attention processor.

Problem (full shapes):
  hidden_states [4, 4096, 1280], encoder_hidden_states [4, 77, 2048],
  id_embedding [2, 32, 2048], Wq/Wout [1280,1280], Wk/Wv/Wid_k/Wid_v
  [2048,1280], bout [1280].  20 heads, dh=64.  Output [4, 4096, 1280].

Sharding: data-parallel over (batch, seq-half): core c handles batch c//2,
query rows (c%2)*2048 : (c%2+1)*2048.  K/V (109 keys) are computed
per-core for its batch.  All queries are independent (full cross
attention), so no collectives are needed.

Per-core pipeline (fp16 matmul operands, fp32 PSUM accumulation):

Phase 1 (DMA-bound, ~100us): q projection in 10 j-slots, with the 21MB
  of K/V projection weights streamed continuously behind it.  One KV
  chunk (512 cols of [k~|v~]) is computed between q-groups at the odd
  slots.  The encoder GEMM (keys 0:77 padded to 0:96) and the id GEMM
  (keys 96:128) are merged into one accumulation pass via column-tiled
  matmuls (col strips 0:64 / 64:96 / 96:128 run concurrently in the PE
  array).  kT is produced from the k columns by DMA transpose.
  A short warmup matmul stream at t=0 lifts the PE HAM clock gate to
  8/8 before real work arrives.

Phase 2 (PE-bound, ~115us): attention and output projection fully
  interleaved.  Per head-pair: 2 row-tiled scores matmuls (K=64, rows
  0:64 / 64:128 concurrent), exp with gap-mask bias on ACT, 2 col-tiled
  PV matmuls + 2 col-tiled ones-denominator matmuls (cols 0:64 / 64:128
  concurrent), reciprocal+normalize on DVE.  attnT is written back into
  the qT tiles (read-then-overwrite per chunk; saves 5MB SBUF).  After
  each chunk's 10 head-pairs finish, its 4 query tiles of the output
  projection are emitted interleaved with the next chunk's attention so
  the exp/normalize work hides under out-proj matmuls and the PE never
  idles.

Key layout: [0:77]=encoder keys, [77:96]=zero gap (exp bias -1e30),
[96:128]=id keys.
"""

import os
import sys
import types

import numpy as np

# ---------------------------------------------------------------------------
# problem constants (hardcoded; kernel.py must be self-contained)
# ---------------------------------------------------------------------------
B = 4
S = 4096
H = 1280
C = 2048
TE = 77          # encoder tokens
TI = 32          # id tokens
HEADS = 20
DH = 64          # head dim
P = 128
L = 109          # TE + TI
LP = 128         # padded key count: [0:77]=ehs, [77:96]=gap, [96:128]=id
GAP0, GAP1 = TE, P - TI   # 77, 96
SC = 2048        # seq rows per core
NJ = H // P      # 10
NI = C // P      # 16
NCH = SC // 512  # 4 sq-chunks of 512
NT = SC // P     # 16 sq-tiles of 128
SCALE = 1.0 / 8.0
NCORES = 8
MCHUNKS = [(0, 512), (512, 512), (1024, 256)]

_NC_CACHE = {}


def _ensure_axon_hooks():
    """The image's antenv lacks axon_hooks; synthesize it so NTFF profiling
    (trace=True) works when test.py asks for it.  Harmless if unused."""
    if "antenv.axon_hooks" in sys.modules:
        return
    try:
        import antenv
        from trn_agent_boot.trn_boot import _ntff_profile_via_ctypes

        hook = _ntff_profile_via_ctypes("/opt/axon/libaxon_pjrt.so")
        m = types.ModuleType("antenv.axon_hooks")
        m.get_axon_ntff_profile_hook = lambda: hook
        m.set_axon_ntff_profile_hook = lambda h: None
        sys.modules["antenv.axon_hooks"] = m
        antenv.axon_hooks = m
    except Exception:
        pass


def build_nc():
    """Build + compile the per-core Bass program (SPMD: same NEFF, 8 cores)."""
    if "nc" in _NC_CACHE:
        return _NC_CACHE["nc"]

    import concourse.bass as bass
    import concourse.tile as tile
    from concourse import bacc, mybir
    from concourse.bass import ts

    F32 = mybir.dt.float32
    R = mybir.dt.float16      # matmul operand dtype (1 cyc/row, 10-bit mantissa)
    EXP = mybir.ActivationFunctionType.Exp

    nc = bacc.Bacc("TRN2", target_bir_lowering=False, debug=False, num_devices=NCORES)

    hsT = nc.dram_tensor("hsT", [H, SC], R, kind="ExternalInput").ap()
    xkvTp = nc.dram_tensor("xkvTp", [P, NI * LP], R, kind="ExternalInput").ap()
    wqp = nc.dram_tensor("wqp", [NJ, P, H], R, kind="ExternalInput").ap()
    # per-core kv weights: even cores get [Wk|Wv], odd cores [Wid_k|Wid_v];
    # each core computes only its projection and the pair exchanges results.
    wkvp = nc.dram_tensor("wkvp", [5, P, NI * 512], R, kind="ExternalInput").ap()
    woutT = nc.dram_tensor("woutT", [H, H], R, kind="ExternalInput").ap()
    boutb = nc.dram_tensor("boutb", [P, H], F32, kind="ExternalInput").ap()
    out = nc.dram_tensor("out", [SC, H], F32, kind="ExternalOutput").ap()
    kvstore = nc.dram_tensor("kvstore", [P, 5 * 512], R, kind="Internal").ap()
    kvgather = nc.dram_tensor("kvgather", [2, P, 5 * 512], R, kind="Internal").ap()

    with tile.TileContext(nc) as tc:
        with tc.tile_pool(name="pers", bufs=1) as pers:
            # ---- persistent constants / arrays --------------------------------
            ones_mat = pers.tile([P, P], R, tag="ones_mat")
            nc.vector.memset(ones_mat[:, :], 1.0)
            bias_col = pers.tile([P, 1], F32, tag="bias_col")
            # engine ops need 32-aligned start partitions: write the gap
            # as [64:96] then restore [64:77]; later writes overwrite cleanly.
            nc.vector.memset(bias_col[:, :], 0.0)
            nc.vector.memset(bias_col[64:GAP1, :], -1e30)
            nc.vector.memset(bias_col[64:GAP0, :], 0.0)
            kT_sb = [pers.tile([P, LP], R, tag=f"kT{j}", name=f"kT{j}") for j in range(NJ)]
            # merged pair-exchanged [k~|v~] (cols 0:1280 k, 1280:2560 v)
            kvall = pers.tile([P, 5 * 512], R, tag="kvall")
            v_sb = kvall[:, 1280:2560]
            # qT doubles as attnT in phase 2 (normalize overwrites each
            # chunk after its scores matmul has consumed it).
            qT_sb = [pers.tile([P, SC], R, tag=f"qT{j}", name=f"qT{j}") for j in range(NJ)]
            wout_sb = [pers.tile([P, H], R, tag=f"wout{i}", name=f"wout{i}") for i in range(NJ)]
            boutb_sb = pers.tile([P, H], F32, tag="boutb")

            # ---- phase 1: q projection + kv projection, DMA-paced -------------
            with (
                tc.tile_pool(name="phq", bufs=1) as phq,
                tc.tile_pool(name="wqs", bufs=3) as wqs,
                tc.tile_pool(name="wkvs", bufs=3) as wkvs,
                tc.tile_pool(name="psq", bufs=5, space="PSUM") as psq,
                tc.tile_pool(name="pskv", bufs=2, space="PSUM") as pskv,
            ):
                hsT_sb = [phq.tile([P, SC], R, tag=f"hsT{i}", name=f"hsT{i}") for i in range(NJ)]
                xkv_all = phq.tile([P, NI * LP], R, tag="xkv_all")
                kvmine = phq.tile([P, 5 * 512], R, tag="kvmine")
                warm_mov = phq.tile([P, 512], R, tag="warm_mov")
                nc.vector.memset(warm_mov[:, :], 0.0)

                # PE warmup: ~10 matmuls on constants so the HAM clock gate
                # reaches 8/8 while the first input DMAs land.  The psum tile
                # is never read; pskv reuse orders it before kv chunk 0.
                wps = pskv.tile([P, 512], F32, tag="kvps", name="warm_ps")
                for w in range(10):
                    nc.tensor.matmul(wps[:, :], ones_mat[:, :], warm_mov[:, :],
                                     start=(w == 0), stop=(w == 9))

                # DMA issue order is arrival order: everything q_group(0)
                # needs goes first.  One mega-DMA per weight group keeps the
                # descriptor stream short and the fan-out wide.
                nc.sync.dma_start(out=hsT_sb[0][:, :], in_=hsT[ts(0, P), :])
                wq_tiles = {}

                def issue_wq(j):
                    t = wqs.tile([P, H], R, tag="wq", name=f"wq{j}")
                    nc.sync.dma_start(out=t[:, :], in_=wqp[j])
                    wq_tiles[j] = t

                issue_wq(0)
                for i in range(1, NJ):
                    nc.sync.dma_start(out=hsT_sb[i][:, :], in_=hsT[ts(i, P), :])
                nc.sync.dma_start(out=xkv_all[:, :], in_=xkvTp)

                # kv sub-chunks: each core computes only ITS projection (even
                # cores [Wk|Wv] over encoder keys, odd cores [Wid_k|Wid_v]
                # over id keys — same code, different weight data) and the
                # pair exchanges results via a 2-core AllGather.
                kv_tiles = {}

                def issue_kv(n):
                    t = wkvs.tile([P, NI * 512], R, tag="wkv", name=f"wkv{n}")
                    nc.sync.dma_start(out=t[:, :], in_=wkvp[n])
                    kv_tiles[n] = t

                def q_group(j):
                    wq_j = wq_tiles[j]
                    pss = [psq.tile([P, 512], F32, tag="qps", name="qps") for _ in range(NCH)]
                    for i in range(NJ):
                        for c in range(NCH):
                            nc.tensor.matmul(
                                pss[c][:, :], wq_j[:, ts(i, P)], hsT_sb[i][:, ts(c, 512)],
                                start=(i == 0), stop=(i == NJ - 1),
                            )
                    for c in range(NCH):
                        nc.scalar.copy(qT_sb[j][:, ts(c, 512)], pss[c][:, :])

                def kv_sub(n):
                    kvw = kv_tiles[n]
                    ps = pskv.tile([P, 512], F32, tag="kvps", name="kvps")
                    for i in range(NI):
                        nc.tensor.matmul(
                            ps[:, :], xkv_all[:, ts(i, LP)], kvw[:, ts(i, 512)],
                            start=(i == 0), stop=(i == NI - 1),
                        )
                    nc.scalar.copy(kvmine[:, ts(n, 512)], ps[:, :])

                issue_kv(0)
                issue_kv(1)
                for j in range(NJ):
                    # prefetch next slot's wq group + the kv sub-chunk two
                    # slots out; wout/bout late in the phase.
                    if j + 1 < NJ:
                        issue_wq(j + 1)
                    if j + 2 < 5:
                        issue_kv(j + 2)
                    if j == 8:
                        for i in range(NJ):
                            nc.sync.dma_start(out=wout_sb[i][:, :], in_=woutT[ts(i, P), :])
                        nc.sync.dma_start(out=boutb_sb[:, :], in_=boutb)
                    q_group(j)
                    if j < 5:
                        kv_sub(j)
                    if j == 4:
                        # my projection done -> store to HBM for the pair
                        nc.sync.dma_start(out=kvstore, in_=kvmine[:, :])
                    if j == 5:
                        # 2-core AllGather: slot 0 = even core ([Wk|Wv] proj,
                        # valid rows 0:96), slot 1 = odd ([Wid_*], rows 96:128)
                        nc.gpsimd.collective_compute(
                            "AllGather", mybir.AluOpType.bypass,
                            replica_groups=[[0, 1], [2, 3], [4, 5], [6, 7]],
                            ins=[kvstore], outs=[kvgather],
                        )
                    if j == 6:
                        nc.sync.dma_start(out=kvall[0:GAP1, :],
                                          in_=kvgather[0, 0:GAP1, :])
                        nc.sync.dma_start(out=kvall[GAP1:P, :],
                                          in_=kvgather[1, GAP1:P, :])
                    # kT transposes from the merged result, late phase 1
                    if 7 <= j <= 8:
                        for t in range(5 * (j - 7), 5 * (j - 7) + 5):
                            nc.sync.dma_start(out=kT_sb[t][:, :],
                                              in_=kvall[:, ts(t, P)], transpose=True)

            # ---- phase 2: attention + output projection, interleaved ----------
            with (
                tc.tile_pool(name="probs", bufs=6) as probs_pool,
                tc.tile_pool(name="bcp", bufs=2) as bc_pool,
                tc.tile_pool(name="finp", bufs=3) as finp,
                tc.tile_pool(name="pss", bufs=3, space="PSUM") as pss,
                tc.tile_pool(name="pso", bufs=2, space="PSUM") as pso,
                tc.tile_pool(name="psd", bufs=1, space="PSUM") as psd,
                tc.tile_pool(name="psf", bufs=2, space="PSUM") as psf,
            ):
                pairs = [(c, hp) for c in range(NCH) for hp in range(NJ)]
                astate = {}

                def attn_front(idx):
                    c, hp = pairs[idx]
                    pts = []
                    for s in range(2):
                        rq = DH * s
                        ps_s = pss.tile([P, 512], F32, tag="sps", name="sps")
                        nc.tensor.matmul(
                            ps_s[:, :], kT_sb[hp][rq:rq + DH, :],
                            qT_sb[hp][rq:rq + DH, ts(c, 512)],
                            start=True, stop=True,
                        )
                        pts.append(ps_s)
                    probs = []
                    for s in range(2):
                        probsT = probs_pool.tile([P, 512], R, tag="probsT", name="probsT")
                        nc.scalar.activation(
                            probsT[:, :], pts[s][:, :], EXP,
                            bias=bias_col[:, :], scale=SCALE,
                        )
                        probs.append(probsT)
                    astate[idx] = probs

                def attn_back(idx):
                    c, hp = pairs[idx]
                    probs = astate.pop(idx)
                    # PV of both heads (disjoint col strips) back-to-back so
                    # they overlap in the array, then both denominators.
                    ps_o = pso.tile([P, 512], F32, tag="ops", name="ops")
                    ps_d = psd.tile([P, 512], F32, tag="dps", name="dps")
                    for s in range(2):
                        h = 2 * hp + s
                        rq = DH * s
                        nc.tensor.matmul(
                            ps_o[rq:rq + DH, :], v_sb[:, ts(h, DH)], probs[s][:, :],
                            start=True, stop=True,
                        )
                    for s in range(2):
                        rq = DH * s
                        nc.tensor.matmul(
                            ps_d[rq:rq + DH, :], ones_mat[:, 0:DH], probs[s][:, :],
                            start=True, stop=True,
                        )
                    bc_sb = bc_pool.tile([P, 512], F32, tag="bc", name="bc_sb")
                    nc.vector.reciprocal_approx_fast(bc_sb[:, :], ps_d[:, :])
                    nc.vector.tensor_mul(
                        qT_sb[hp][:, ts(c, 512)], ps_o[:, :], bc_sb[:, :]
                    )

                def out_tile(t):
                    fin = finp.tile([P, H], F32, tag="fin", name="fin")
                    for m0, mw in MCHUNKS:
                        pf = psf.tile([P, mw], F32, tag="psf", name="psf")
                        for i in range(NJ):
                            nc.tensor.matmul(
                                pf[:, :], qT_sb[i][:, ts(t, P)],
                                wout_sb[i][:, m0:m0 + mw],
                                start=(i == 0), stop=(i == NJ - 1),
                            )
                        nc.vector.tensor_add(
                            fin[:, m0:m0 + mw], pf[:, :], boutb_sb[:, m0:m0 + mw]
                        )
                    nc.sync.dma_start(out=out[ts(t, P), :], in_=fin[:, :])

                ready_tiles = []
                done_tiles = 0
                for idx in range(len(pairs)):
                    attn_front(idx)
                    if idx >= 1:
                        attn_back(idx - 1)
                        pc, php = pairs[idx - 1]
                        if php == NJ - 1:
                            ready_tiles.extend(range(4 * pc, 4 * pc + 4))
                    if ready_tiles:
                        out_tile(ready_tiles.pop(0))
                        done_tiles += 1
                attn_back(len(pairs) - 1)
                ready_tiles.extend(range(12, 16))
                for t in ready_tiles:
                    out_tile(t)

    nc.compile()
    _NC_CACHE["nc"] = nc
    return nc


def prep_core_inputs(hidden_states, encoder_hidden_states, id_embedding,
                     Wq, Wk, Wv, Wid_k, Wid_v, Wout, bout):
    """Host-side sharding / layout prep.  Returns list of 8 in_maps."""
    f = np.float32
    h16 = np.float16
    hidden_states = np.asarray(hidden_states, f)
    encoder_hidden_states = np.asarray(encoder_hidden_states, f)
    id_embedding = np.asarray(id_embedding, f)
    Wq = np.asarray(Wq, f)
    Wout = np.asarray(Wout, f)
    Wk, Wv = np.asarray(Wk, f), np.asarray(Wv, f)
    Wid_k, Wid_v = np.asarray(Wid_k, f), np.asarray(Wid_v, f)
    boutb = np.ascontiguousarray(np.broadcast_to(np.asarray(bout, f), (P, H)))

    # packed mega-tile weight layouts: one contiguous DMA per group, with
    # [128-partition, i-major] free dims so per-i slices are plain column
    # ranges in SBUF.
    wqp = np.ascontiguousarray(
        Wq.reshape(NJ, P, NJ, P).transpose(2, 1, 0, 3).reshape(NJ, P, H)
        .astype(h16))                                                          # [j][p, i*128+m]
    wkv = np.concatenate([Wk, Wv], axis=1)                                     # [C, 2H]
    widkv = np.concatenate([Wid_k, Wid_v], axis=1)
    wkvp = np.ascontiguousarray(
        wkv.reshape(NI, P, 5, 512).transpose(2, 1, 0, 3).reshape(5, P, NI * 512)
        .astype(h16))                                                          # [n][p, i*512+m]
    widkvp = np.ascontiguousarray(
        widkv.reshape(NI, P, 5, 512).transpose(2, 1, 0, 3).reshape(5, P, NI * 512)
        .astype(h16))
    # pair-split: even core streams the encoder projection weights, odd core
    # the id projection weights; results are exchanged on-device.

    wout16 = np.ascontiguousarray(Wout.astype(h16))
    in_maps = []
    for core in range(NCORES):
        b, hf = divmod(core, 2)
        hsT = np.ascontiguousarray(hidden_states[b, hf * SC:(hf + 1) * SC, :].T.astype(h16))
        xkvT = np.zeros((C, LP), h16)                                          # [C, 128]
        xkvT[:, :TE] = encoder_hidden_states[b].T
        xkvT[:, GAP1:] = id_embedding[b % 2].T
        xkvTp = np.ascontiguousarray(
            xkvT.reshape(NI, P, LP).transpose(1, 0, 2).reshape(P, NI * LP))    # [p, i*128+l]
        in_maps.append({
            "hsT": hsT, "xkvTp": xkvTp, "wqp": wqp,
            "wkvp": wkvp if core % 2 == 0 else widkvp,
            "woutT": wout16, "boutb": boutb,
        })
    return in_maps


def kernel(hidden_states, encoder_hidden_states, id_embedding,
           Wq, Wk, Wv, Wid_k, Wid_v, Wout, bout, _trace=False):
    _ensure_axon_hooks()
    from concourse.bass_utils import run_bass_kernel_spmd

    nc = build_nc()
    in_maps = prep_core_inputs(hidden_states, encoder_hidden_states, id_embedding,
                               Wq, Wk, Wv, Wid_k, Wid_v, Wout, bout)
    kwargs = {}
    if _trace:
        import concourse.bass_utils as bu
        bu.upload_artifacts = lambda tmpdir: f"local://{tmpdir}"
        kwargs["trace"] = True
    res = run_bass_kernel_spmd(nc, in_maps, core_ids=list(range(NCORES)), **kwargs)

    outp = np.empty((B, S, H), np.float32)
    for core in range(NCORES):
        b, hf = divmod(core, 2)
        outp[b, hf * SC:(hf + 1) * SC, :] = res.results[core]["out"]
    if _trace:
        kernel.last_exec_time_ns = res.exec_time_ns
        kernel.last_results = res
    return outp


# revision 26
# speedup vs baseline: 1.1681x; 1.1681x over previous
"""Trainium2 Bass kernel for the branched cross-attention processor.

Problem (full shapes):
  hidden_states [4, 4096, 1280], encoder_hidden_states [4, 77, 2048],
  id_embedding [2, 32, 2048], Wq/Wout [1280,1280], Wk/Wv/Wid_k/Wid_v
  [2048,1280], bout [1280].  20 heads, dh=64.  Output [4, 4096, 1280].

Sharding: data-parallel over (batch, seq-half): core c handles batch c//2,
query rows (c%2)*2048 : (c%2+1)*2048.  K/V (109 keys) are computed
per-core for its batch.  All queries are independent (full cross
attention), so no collectives are needed.

Per-core pipeline (fp16 matmul operands, fp32 PSUM accumulation):

Phase 1 (DMA-bound, ~100us): q projection in 10 j-slots, with the 21MB
  of K/V projection weights streamed continuously behind it.  One KV
  chunk (512 cols of [k~|v~]) is computed between q-groups at the odd
  slots.  The encoder GEMM (keys 0:77 padded to 0:96) and the id GEMM
  (keys 96:128) are merged into one accumulation pass via column-tiled
  matmuls (col strips 0:64 / 64:96 / 96:128 run concurrently in the PE
  array).  kT is produced from the k columns by DMA transpose.
  A short warmup matmul stream at t=0 lifts the PE HAM clock gate to
  8/8 before real work arrives.

Phase 2 (PE-bound, ~115us): attention and output projection fully
  interleaved.  Per head-pair: 2 row-tiled scores matmuls (K=64, rows
  0:64 / 64:128 concurrent), exp with gap-mask bias on ACT, 2 col-tiled
  PV matmuls + 2 col-tiled ones-denominator matmuls (cols 0:64 / 64:128
  concurrent), reciprocal+normalize on DVE.  attnT is written back into
  the qT tiles (read-then-overwrite per chunk; saves 5MB SBUF).  After
  each chunk's 10 head-pairs finish, its 4 query tiles of the output
  projection are emitted interleaved with the next chunk's attention so
  the exp/normalize work hides under out-proj matmuls and the PE never
  idles.

Key layout: [0:77]=encoder keys, [77:96]=zero gap (exp bias -1e30),
[96:128]=id keys.
"""

import os
import sys
import types

import numpy as np

# ---------------------------------------------------------------------------
# problem constants (hardcoded; kernel.py must be self-contained)
# ---------------------------------------------------------------------------
B = 4
S = 4096
H = 1280
C = 2048
TE = 77          # encoder tokens
TI = 32          # id tokens
HEADS = 20
DH = 64          # head dim
P = 128
L = 109          # TE + TI
LP = 128         # padded key count: [0:77]=ehs, [77:96]=gap, [96:128]=id
GAP0, GAP1 = TE, P - TI   # 77, 96
SC = 2048        # seq rows per core
NJ = H // P      # 10
NI = C // P      # 16
NCH = SC // 512  # 4 sq-chunks of 512
NT = SC // P     # 16 sq-tiles of 128
SCALE = 1.0 / 8.0
NCORES = 8
MCHUNKS = [(0, 512), (512, 512), (1024, 256)]

_NC_CACHE = {}


def _ensure_axon_hooks():
    """The image's antenv lacks axon_hooks; synthesize it so NTFF profiling
    (trace=True) works when test.py asks for it.  Harmless if unused."""
    if "antenv.axon_hooks" in sys.modules:
        return
    try:
        import antenv
        from trn_agent_boot.trn_boot import _ntff_profile_via_ctypes

        hook = _ntff_profile_via_ctypes("/opt/axon/libaxon_pjrt.so")
        m = types.ModuleType("antenv.axon_hooks")
        m.get_axon_ntff_profile_hook = lambda: hook
        m.set_axon_ntff_profile_hook = lambda h: None
        sys.modules["antenv.axon_hooks"] = m
        antenv.axon_hooks = m
    except Exception:
        pass


def build_nc():
    """Build + compile the per-core Bass program (SPMD: same NEFF, 8 cores)."""
    if "nc" in _NC_CACHE:
        return _NC_CACHE["nc"]

    import concourse.bass as bass
    import concourse.tile as tile
    from concourse import bacc, mybir
    from concourse.bass import ts

    F32 = mybir.dt.float32
    R = mybir.dt.float16      # matmul operand dtype (1 cyc/row, 10-bit mantissa)
    EXP = mybir.ActivationFunctionType.Exp

    nc = bacc.Bacc("TRN2", target_bir_lowering=False, debug=False, num_devices=NCORES)

    hsT = nc.dram_tensor("hsT", [H, SC], R, kind="ExternalInput").ap()
    xkvTp = nc.dram_tensor("xkvTp", [P, NI * LP], R, kind="ExternalInput").ap()
    wqp = nc.dram_tensor("wqp", [NJ, P, H], R, kind="ExternalInput").ap()
    # per-core kv weights: even cores get [Wk|Wv], odd cores [Wid_k|Wid_v];
    # each core computes only its projection and the pair exchanges results.
    wkvp = nc.dram_tensor("wkvp", [5, P, NI * 512], R, kind="ExternalInput").ap()
    woutT = nc.dram_tensor("woutT", [H, H], R, kind="ExternalInput").ap()
    boutb = nc.dram_tensor("boutb", [P, H], F32, kind="ExternalInput").ap()
    out = nc.dram_tensor("out", [SC, H], F32, kind="ExternalOutput").ap()
    kvstore = nc.dram_tensor("kvstore", [P, 5 * 512], R, kind="Internal").ap()
    kvgather = nc.dram_tensor("kvgather", [2, P, 5 * 512], R, kind="Internal").ap()

    with tile.TileContext(nc) as tc:
        with tc.tile_pool(name="pers", bufs=1) as pers:
            # ---- persistent constants / arrays --------------------------------
            ones_mat = pers.tile([P, P], R, tag="ones_mat")
            nc.vector.memset(ones_mat[:, :], 1.0)
            bias_col = pers.tile([P, 1], F32, tag="bias_col")
            # engine ops need 32-aligned start partitions: write the gap
            # as [64:96] then restore [64:77]; later writes overwrite cleanly.
            nc.vector.memset(bias_col[:, :], 0.0)
            nc.vector.memset(bias_col[64:GAP1, :], -1e30)
            nc.vector.memset(bias_col[64:GAP0, :], 0.0)
            kT_sb = [pers.tile([P, LP], R, tag=f"kT{j}", name=f"kT{j}") for j in range(NJ)]
            # merged pair-exchanged [k~|v~] (cols 0:1280 k, 1280:2560 v)
            kvall = pers.tile([P, 5 * 512], R, tag="kvall")
            v_sb = kvall[:, 1280:2560]
            # qT doubles as attnT in phase 2 (normalize overwrites each
            # chunk after its scores matmul has consumed it).
            qT_sb = [pers.tile([P, SC], R, tag=f"qT{j}", name=f"qT{j}") for j in range(NJ)]
            wout_sb = [pers.tile([P, H], R, tag=f"wout{i}", name=f"wout{i}") for i in range(NJ)]
            boutb_sb = pers.tile([P, H], F32, tag="boutb")

            # ---- phase 1: q projection + kv projection, DMA-paced -------------
            with (
                tc.tile_pool(name="phq", bufs=1) as phq,
                tc.tile_pool(name="wqs", bufs=3) as wqs,
                tc.tile_pool(name="wkvs", bufs=3) as wkvs,
                tc.tile_pool(name="psq", bufs=5, space="PSUM") as psq,
                tc.tile_pool(name="pskv", bufs=2, space="PSUM") as pskv,
            ):
                hsT_sb = [phq.tile([P, SC], R, tag=f"hsT{i}", name=f"hsT{i}") for i in range(NJ)]
                xkv_all = phq.tile([P, NI * LP], R, tag="xkv_all")
                kvmine = phq.tile([P, 5 * 512], R, tag="kvmine")
                warm_mov = phq.tile([P, 512], R, tag="warm_mov")
                nc.vector.memset(warm_mov[:, :], 0.0)

                # PE warmup: ~10 matmuls on constants so the HAM clock gate
                # reaches 8/8 while the first input DMAs land.  The psum tile
                # is never read; pskv reuse orders it before kv chunk 0.
                wps = pskv.tile([P, 512], F32, tag="kvps", name="warm_ps")
                for w in range(10):
                    nc.tensor.matmul(wps[:, :], ones_mat[:, :], warm_mov[:, :],
                                     start=(w == 0), stop=(w == 9))

                # DMA issue order is arrival order: everything q_group(0)
                # needs goes first.  One mega-DMA per weight group keeps the
                # descriptor stream short and the fan-out wide.
                nc.sync.dma_start(out=hsT_sb[0][:, :], in_=hsT[ts(0, P), :])
                wq_tiles = {}

                def issue_wq(j):
                    t = wqs.tile([P, H], R, tag="wq", name=f"wq{j}")
                    nc.sync.dma_start(out=t[:, :], in_=wqp[j])
                    wq_tiles[j] = t

                issue_wq(0)
                for i in range(1, NJ):
                    nc.sync.dma_start(out=hsT_sb[i][:, :], in_=hsT[ts(i, P), :])
                nc.sync.dma_start(out=xkv_all[:, :], in_=xkvTp)

                # kv sub-chunks: each core computes only ITS projection (even
                # cores [Wk|Wv] over encoder keys, odd cores [Wid_k|Wid_v]
                # over id keys — same code, different weight data) and the
                # pair exchanges results via a 2-core AllGather.
                kv_tiles = {}

                def issue_kv(n):
                    t = wkvs.tile([P, NI * 512], R, tag="wkv", name=f"wkv{n}")
                    nc.sync.dma_start(out=t[:, :], in_=wkvp[n])
                    kv_tiles[n] = t

                def q_group(j):
                    wq_j = wq_tiles[j]
                    pss = [psq.tile([P, 512], F32, tag="qps", name="qps") for _ in range(NCH)]
                    for i in range(NJ):
                        for c in range(NCH):
                            nc.tensor.matmul(
                                pss[c][:, :], wq_j[:, ts(i, P)], hsT_sb[i][:, ts(c, 512)],
                                start=(i == 0), stop=(i == NJ - 1),
                            )
                    for c in range(NCH):
                        nc.scalar.copy(qT_sb[j][:, ts(c, 512)], pss[c][:, :])

                def kv_sub(n):
                    kvw = kv_tiles[n]
                    ps = pskv.tile([P, 512], F32, tag="kvps", name="kvps")
                    for i in range(NI):
                        nc.tensor.matmul(
                            ps[:, :], xkv_all[:, ts(i, LP)], kvw[:, ts(i, 512)],
                            start=(i == 0), stop=(i == NI - 1),
                        )
                    nc.scalar.copy(kvmine[:, ts(n, 512)], ps[:, :])

                issue_kv(0)
                issue_kv(1)
                for j in range(NJ):
                    # prefetch next slot's wq group + the kv sub-chunk two
                    # slots out; wout/bout late in the phase.
                    if j + 1 < NJ:
                        issue_wq(j + 1)
                    if j + 2 < 5:
                        issue_kv(j + 2)
                    if j == 8:
                        for i in range(NJ):
                            nc.sync.dma_start(out=wout_sb[i][:, :], in_=woutT[ts(i, P), :])
                        nc.sync.dma_start(out=boutb_sb[:, :], in_=boutb)
                    q_group(j)
                    if j < 5:
                        kv_sub(j)
                    if j == 4:
                        # my projection done -> store to HBM for the pair
                        nc.sync.dma_start(out=kvstore, in_=kvmine[:, :])
                    if j == 5:
                        # 2-core AllGather: slot 0 = even core ([Wk|Wv] proj,
                        # valid rows 0:96), slot 1 = odd ([Wid_*], rows 96:128)
                        nc.gpsimd.collective_compute(
                            "AllGather", mybir.AluOpType.bypass,
                            replica_groups=[[0, 1], [2, 3], [4, 5], [6, 7]],
                            ins=[kvstore], outs=[kvgather],
                        )
                    if j == 6:
                        nc.sync.dma_start(out=kvall[0:GAP1, :],
                                          in_=kvgather[0, 0:GAP1, :])
                        nc.sync.dma_start(out=kvall[GAP1:P, :],
                                          in_=kvgather[1, GAP1:P, :])
                    # kT transposes from the merged result, late phase 1
                    if 7 <= j <= 8:
                        for t in range(5 * (j - 7), 5 * (j - 7) + 5):
                            nc.sync.dma_start(out=kT_sb[t][:, :],
                                              in_=kvall[:, ts(t, P)], transpose=True)

            # ---- phase 2: attention + output projection, interleaved ----------
            with (
                tc.tile_pool(name="probs", bufs=6) as probs_pool,
                tc.tile_pool(name="bcp", bufs=2) as bc_pool,
                tc.tile_pool(name="finp", bufs=3) as finp,
                tc.tile_pool(name="pss", bufs=2, space="PSUM") as pss,
                tc.tile_pool(name="pso", bufs=1, space="PSUM") as pso,
                tc.tile_pool(name="psd", bufs=1, space="PSUM") as psd,
                tc.tile_pool(name="psf", bufs=2, space="PSUM") as psf,
            ):
                pairs = [(c, hp) for c in range(NCH) for hp in range(NJ)]
                astate = {}

                def attn_front(idx):
                    c, hp = pairs[idx]
                    # both heads' scores in one 2-bank psum tile: one WAR
                    # wait for the pair, so the two row-group matmuls can
                    # overlap in the array.
                    ps_s = pss.tile([P, 1024], F32, tag="sps", name="sps")
                    for s in range(2):
                        rq = DH * s
                        nc.tensor.matmul(
                            ps_s[:, ts(s, 512)], kT_sb[hp][rq:rq + DH, :],
                            qT_sb[hp][rq:rq + DH, ts(c, 512)],
                            start=True, stop=True,
                        )
                    probs = []
                    for s in range(2):
                        probsT = probs_pool.tile([P, 512], R, tag="probsT", name="probsT")
                        nc.scalar.activation(
                            probsT[:, :], ps_s[:, ts(s, 512)], EXP,
                            bias=bias_col[:, :], scale=SCALE,
                        )
                        probs.append(probsT)
                    astate[idx] = probs

                def attn_back(idx):
                    c, hp = pairs[idx]
                    probs = astate.pop(idx)
                    # PV of both heads (disjoint col strips) back-to-back so
                    # they overlap in the array, then both denominators.
                    ps_o = pso.tile([P, 512], F32, tag="ops", name="ops")
                    ps_d = psd.tile([P, 512], F32, tag="dps", name="dps")
                    for s in range(2):
                        h = 2 * hp + s
                        rq = DH * s
                        nc.tensor.matmul(
                            ps_o[rq:rq + DH, :], v_sb[:, ts(h, DH)], probs[s][:, :],
                            start=True, stop=True,
                        )
                    for s in range(2):
                        rq = DH * s
                        nc.tensor.matmul(
                            ps_d[rq:rq + DH, :], ones_mat[:, 0:DH], probs[s][:, :],
                            start=True, stop=True,
                        )
                    bc_sb = bc_pool.tile([P, 512], F32, tag="bc", name="bc_sb")
                    nc.vector.reciprocal_approx_fast(bc_sb[:, :], ps_d[:, :])
                    nc.vector.tensor_mul(
                        qT_sb[hp][:, ts(c, 512)], ps_o[:, :], bc_sb[:, :]
                    )

                def out_tile(t):
                    fin = finp.tile([P, H], F32, tag="fin", name="fin")
                    for m0, mw in MCHUNKS:
                        pf = psf.tile([P, mw], F32, tag="psf", name="psf")
                        for i in range(NJ):
                            nc.tensor.matmul(
                                pf[:, :], qT_sb[i][:, ts(t, P)],
                                wout_sb[i][:, m0:m0 + mw],
                                start=(i == 0), stop=(i == NJ - 1),
                            )
                        nc.vector.tensor_add(
                            fin[:, m0:m0 + mw], pf[:, :], boutb_sb[:, m0:m0 + mw]
                        )
                    nc.sync.dma_start(out=out[ts(t, P), :], in_=fin[:, :])

                ready_tiles = []
                done_tiles = 0
                for idx in range(len(pairs)):
                    attn_front(idx)
                    if idx >= 1:
                        attn_back(idx - 1)
                        pc, php = pairs[idx - 1]
                        if php == NJ - 1:
                            ready_tiles.extend(range(4 * pc, 4 * pc + 4))
                    if ready_tiles:
                        out_tile(ready_tiles.pop(0))
                        done_tiles += 1
                attn_back(len(pairs) - 1)
                ready_tiles.extend(range(12, 16))
                for t in ready_tiles:
                    out_tile(t)

    nc.compile()
    _NC_CACHE["nc"] = nc
    return nc


def prep_core_inputs(hidden_states, encoder_hidden_states, id_embedding,
                     Wq, Wk, Wv, Wid_k, Wid_v, Wout, bout):
    """Host-side sharding / layout prep.  Returns list of 8 in_maps."""
    f = np.float32
    h16 = np.float16
    hidden_states = np.asarray(hidden_states, f)
    encoder_hidden_states = np.asarray(encoder_hidden_states, f)
    id_embedding = np.asarray(id_embedding, f)
    Wq = np.asarray(Wq, f)
    Wout = np.asarray(Wout, f)
    Wk, Wv = np.asarray(Wk, f), np.asarray(Wv, f)
    Wid_k, Wid_v = np.asarray(Wid_k, f), np.asarray(Wid_v, f)
    boutb = np.ascontiguousarray(np.broadcast_to(np.asarray(bout, f), (P, H)))

    # packed mega-tile weight layouts: one contiguous DMA per group, with
    # [128-partition, i-major] free dims so per-i slices are plain column
    # ranges in SBUF.
    wqp = np.ascontiguousarray(
        Wq.reshape(NJ, P, NJ, P).transpose(2, 1, 0, 3).reshape(NJ, P, H)
        .astype(h16))                                                          # [j][p, i*128+m]
    wkv = np.concatenate([Wk, Wv], axis=1)                                     # [C, 2H]
    widkv = np.concatenate([Wid_k, Wid_v], axis=1)
    wkvp = np.ascontiguousarray(
        wkv.reshape(NI, P, 5, 512).transpose(2, 1, 0, 3).reshape(5, P, NI * 512)
        .astype(h16))                                                          # [n][p, i*512+m]
    widkvp = np.ascontiguousarray(
        widkv.reshape(NI, P, 5, 512).transpose(2, 1, 0, 3).reshape(5, P, NI * 512)
        .astype(h16))
    # pair-split: even core streams the encoder projection weights, odd core
    # the id projection weights; results are exchanged on-device.

    wout16 = np.ascontiguousarray(Wout.astype(h16))
    in_maps = []
    for core in range(NCORES):
        b, hf = divmod(core, 2)
        hsT = np.ascontiguousarray(hidden_states[b, hf * SC:(hf + 1) * SC, :].T.astype(h16))
        xkvT = np.zeros((C, LP), h16)                                          # [C, 128]
        xkvT[:, :TE] = encoder_hidden_states[b].T
        xkvT[:, GAP1:] = id_embedding[b % 2].T
        xkvTp = np.ascontiguousarray(
            xkvT.reshape(NI, P, LP).transpose(1, 0, 2).reshape(P, NI * LP))    # [p, i*128+l]
        in_maps.append({
            "hsT": hsT, "xkvTp": xkvTp, "wqp": wqp,
            "wkvp": wkvp if core % 2 == 0 else widkvp,
            "woutT": wout16, "boutb": boutb,
        })
    return in_maps


def kernel(hidden_states, encoder_hidden_states, id_embedding,
           Wq, Wk, Wv, Wid_k, Wid_v, Wout, bout, _trace=False):
    _ensure_axon_hooks()
    from concourse.bass_utils import run_bass_kernel_spmd

    nc = build_nc()
    in_maps = prep_core_inputs(hidden_states, encoder_hidden_states, id_embedding,
                               Wq, Wk, Wv, Wid_k, Wid_v, Wout, bout)
    kwargs = {}
    if _trace:
        import concourse.bass_utils as bu
        bu.upload_artifacts = lambda tmpdir: f"local://{tmpdir}"
        kwargs["trace"] = True
    res = run_bass_kernel_spmd(nc, in_maps, core_ids=list(range(NCORES)), **kwargs)

    outp = np.empty((B, S, H), np.float32)
    for core in range(NCORES):
        b, hf = divmod(core, 2)
        outp[b, hf * SC:(hf + 1) * SC, :] = res.results[core]["out"]
    if _trace:
        kernel.last_exec_time_ns = res.exec_time_ns
        kernel.last_results = res
    return outp
